# revision 14
# baseline (speedup 1.0000x reference)
"""DND-LSTM cell, distributed over 8 TRN2 NeuronCores.

Sharding: the episodic dictionary (keys/vals, [100000, 256]) is split into 8
row-shards of 12500. Each core streams its keys shard from HBM, computes
unnormalized cosine-softmax weights u_i = exp((k_i . qhat)/||k_i||), a local
partial retrieval r_loc = sum_i u_i * vals_i (PE matmuls into PSUM) and the
local partition function s_loc = sum_i u_i. One tiny (264-float) AllReduce
produces the global (r, s); every core then finishes the LSTM cell + A2C head
redundantly. The LSTM gate matvecs (Wi/Wh) are computed on-device, overlapped
with the keys/vals streaming.

The new_keys/new_vals outputs equal the inputs except for one row
(write_idx), whose new content (h_t, c_t) is part of out_vec; the row update
is applied host-side after the gather.
"""

import numpy as np

import concourse.bass as bass
import concourse.bacc as bacc
import concourse.mybir as mybir
import concourse.tile as tile
from concourse.bass_utils import run_bass_kernel_spmd

F32 = mybir.dt.float32
BF16 = mybir.dt.bfloat16
AF = mybir.ActivationFunctionType
ALU = mybir.AluOpType
AX = mybir.AxisListType

H = 256
NG = 4
OUT = 8
DICT = 100000
N_CORES = 8
SHARD = DICT // N_CORES  # 12500
OUTW = 2 * H + OUT + 1  # 521


def _body(nc, tc, shard_rows, n_cores, t):
    persist = tc.alloc_tile_pool(name="persist", bufs=1)
    persistB = tc.alloc_tile_pool(name="persistB", bufs=1)
    persist_psum = tc.alloc_tile_pool(name="persist_psum", bufs=1, space="PSUM")
    persist_dram = tc.alloc_tile_pool(name="persist_dram", bufs=1, space="DRAM")

    def T(shape, name, space="SBUF", addr_space="Local"):
        pool = {"SBUF": persist, "PSUM": persist_psum, "DRAM": persist_dram}[space]
        return pool.tile(shape, F32, name=name, tag=name, addr_space=addr_space)

    try:
        _body_inner(nc, tc, shard_rows, n_cores, t, T, persistB)
    finally:
        persist_dram.release()
        persist_psum.release()
        persistB.release()
        persist.release()


def _body_inner(nc, tc, shard_rows, n_cores, t, T, persistB):

    n_full = shard_rows // 128
    rem = shard_rows % 128
    nch = n_full + (1 if rem else 0)
    ngrp = n_full // 8
    lf = n_full % 8

    keys, vals = t["keys"].ap(), t["vals"].ap()
    x_in, h_in, c_in = t["x_in"].ap(), t["h_in"].ap(), t["c_in"].ap()
    Wi_in, Wh_in, bias_in = t["Wi_in"].ap(), t["Wh_in"].ap(), t["bias_in"].ap()
    Wa_in, ba_in = t["Wa_in"].ap(), t["ba_in"].ap()
    Wc_in, bc_in = t["Wc_in"].ap(), t["bc_in"].ap()
    out_vec = t["out_vec"].ap()

    # ---- constants / small inputs ----
    ones_row = T([1, 128], "ones_row")
    nc.vector.memset(ones_row[:], 1.0)
    ones_col = T([128, 1], "ones_col")
    nc.vector.memset(ones_col[:], 1.0)

    x_row = T([1, 256], "x_row")
    nc.sync.dma_start(out=x_row[:], in_=x_in[:])
    h_row0 = T([1, 256], "h_row0")
    nc.sync.dma_start(out=h_row0[:], in_=h_in[:])
    cprev = T([128, 2], "cprev")
    nc.sync.dma_start(out=cprev[:], in_=c_in[:].rearrange("(j p) -> p j", p=128))

    biasc = T([128, 10], "biasc")
    nc.sync.dma_start(out=biasc[:], in_=bias_in[:].rearrange("(t p) -> p t", p=128))
    Wa_sb = T([8, 256], "Wa_sb")
    nc.sync.dma_start(out=Wa_sb[:], in_=Wa_in[:, :])
    ba_sb = T([8, 1], "ba_sb")
    nc.sync.dma_start(out=ba_sb[:], in_=ba_in[:])
    Wc_sb = T([1, 256], "Wc_sb")
    nc.sync.dma_start(out=Wc_sb[:], in_=Wc_in[:, :])
    bc_sb = T([1, 1], "bc_sb")
    nc.sync.dma_start(out=bc_sb[:], in_=bc_in[:])

    # ---- query normalization: qhat = h / ||h|| ----
    scr_h = T([1, 256], "scr_h")
    hh = T([1, 1], "hh")
    nc.scalar.activation(out=scr_h[:], in_=h_row0[:], func=AF.Square,
                         accum_out=hh[:])
    hnorm = T([1, 1], "hnorm")
    nc.scalar.activation(out=hnorm[:], in_=hh[:], func=AF.Sqrt)
    hinv = T([1, 1], "hinv")
    nc.vector.reciprocal(out=hinv[:], in_=hnorm[:])
    qn_row = T([1, 256], "qn_row")
    nc.vector.tensor_scalar_mul(qn_row[:], h_row0[:], hinv[0:1, 0:1])

    # ---- broadcasts to 128 partitions (via K=1 matmul with ones) ----
    with tc.tile_pool(name="psbc", bufs=2, space="PSUM") as psbc:
        pq = psbc.tile([128, 256], F32, name="pq")
        nc.tensor.matmul(pq[:], ones_row[:], qn_row[:])
        qn_b = T([128, 256], "qn_b")
        nc.vector.tensor_copy(qn_b[:], pq[:])

        px = psbc.tile([128, 256], F32, name="px")
        nc.tensor.matmul(px[:], ones_row[:], x_row[:])
        x_b = T([128, 256], "x_b")
        nc.vector.tensor_copy(x_b[:], px[:])

        ph = psbc.tile([128, 256], F32, name="ph")
        nc.tensor.matmul(ph[:], ones_row[:], h_row0[:])
        h_b = T([128, 256], "h_b")
        nc.vector.tensor_copy(h_b[:], ph[:])

    # ---- -0.5 * ||qhat||^2 broadcast (for the polarization identity) ----
    scr_q = T([1, 256], "scr_q")
    hqq = T([1, 1], "hqq")
    nc.scalar.activation(out=scr_q[:], in_=qn_row[:], func=AF.Square,
                         accum_out=hqq[:])
    with tc.tile_pool(name="psq", bufs=1, space="PSUM") as psq:
        pmh = psq.tile([128, 256], F32, name="pmh")
        nc.tensor.matmul(pmh[0:128, 0:1], ones_row[:], hqq[:])
        mh_bc = T([128, 1], "mh_bc")
        nc.vector.tensor_scalar_mul(mh_bc[:], pmh[0:128, 0:1], -0.5)

    # ---- LSTM gate pre-activations (overlaps the main stream) ----
    pre1 = T([128, 10], "pre1")
    pre2 = T([128, 10], "pre2")
    with tc.tile_pool(name="scrw", bufs=3) as scrw:
        Wi_sb = scrw.tile([128, 10, 256], F32, name="Wi_sb", tag="Wi_sb")
        nc.sync.dma_start(out=Wi_sb[:], in_=Wi_in[:, :].rearrange("(t p) h -> p t h", p=128))
        Wh_sb = scrw.tile([128, 10, 256], F32, name="Wh_sb", tag="Wh_sb")
        nc.sync.dma_start(out=Wh_sb[:], in_=Wh_in[:, :].rearrange("(t p) h -> p t h", p=128))
        for ti in range(10):
            s1 = scrw.tile([128, 256], F32, name=f"s1_{ti}", tag="scrw")
            nc.vector.tensor_mul(s1[:], Wi_sb[:, ti, :], x_b[:])
            nc.vector.reduce_sum(pre1[:, ti:ti + 1], s1[:], axis=AX.X)
            s2 = scrw.tile([128, 256], F32, name=f"s2_{ti}", tag="scrw")
            nc.vector.tensor_mul(s2[:], Wh_sb[:, ti, :], h_b[:])
            nc.vector.reduce_sum(pre2[:, ti:ti + 1], s2[:], axis=AX.X)
    pre_s = T([128, 10], "pre_s")
    nc.vector.tensor_add(pre_s[:], pre1[:], pre2[:])
    pre_b = T([128, 10], "pre_b")
    nc.vector.tensor_add(pre_b[:], pre_s[:], biasc[:])
    gsb = T([128, 8], "gsb")
    nc.scalar.activation(out=gsb[:], in_=pre_b[:, 0:8], func=AF.Sigmoid)
    cnsb = T([128, 2], "cnsb")
    nc.scalar.activation(out=cnsb[:], in_=pre_b[:, 8:10], func=AF.Tanh)

    # ---- main stream over the dictionary shard ----
    # polarization identity: k . qhat = (|k + qhat|^2 - |k|^2 - |qhat|^2) / 2
    # -> both row-reduces are ACT Square+accum (single warm LUT function);
    # DVE only does one tensor_add per chunk. The retrieval matmul runs in
    # bf16 (vals cast on the otherwise-idle GpSimd, exp writes u as bf16).
    s1 = T([128, nch], "s1")
    ssq = T([128, nch], "ssq")
    num = T([128, nch], "num")
    u_bf = persistB.tile([128, nch], BF16, name="u_bf", tag="u_bf")
    # pad lanes of the partial last chunk: s1=-199, ssq=1 -> num=-100.5 ->
    # u = exp(-100.5) == 0, contributing nothing to r or s
    nc.vector.memset(s1[:], -199.0)
    nc.vector.memset(ssq[:], 1.0)

    psum_r = T([1, 256], "psum_r", space="PSUM")

    with (
        tc.tile_pool(name="kt", bufs=5) as kpool,
        tc.tile_pool(name="vt", bufs=4) as vpool,
        tc.tile_pool(name="vbf", bufs=7) as vbfpool,
        tc.tile_pool(name="sadd", bufs=3) as saddp,
        tc.tile_pool(name="small", bufs=3) as small,
        tc.tile_pool(name="scra", bufs=4) as scra,
    ):
        def chunk_sim(kt, b, c, r):
            sadd = saddp.tile([128, 256], F32, name=f"sd{c}", tag="sadd")
            nc.vector.tensor_add(sadd[0:r, :], kt[0:r, b, :], qn_b[0:r, :])
            sa = scra.tile([128, 256], F32, name=f"sa{c}", tag="scra")
            nc.scalar.activation(out=sa[0:r, :], in_=sadd[0:r, :],
                                 func=AF.Square, accum_out=s1[0:r, c:c + 1])
            sb = scra.tile([128, 256], F32, name=f"sb{c}", tag="scra")
            nc.scalar.activation(out=sb[0:r, :], in_=kt[0:r, b, :],
                                 func=AF.Square, accum_out=ssq[0:r, c:c + 1])

        def super_u(c0, c1):
            # u[:, c0:c1] = exp(num / sqrt(ssq)), batched to amortize the
            # ACT LUT switches (Sqrt, Exp) across a supergroup
            w = c1 - c0
            dd = small.tile([128, w], F32, name=f"dd{c0}", tag="dd")
            nc.vector.tensor_sub(dd[:], s1[:, c0:c1], ssq[:, c0:c1])
            nc.vector.tensor_scalar(
                out=num[:, c0:c1], in0=dd[:], scalar1=0.5,
                scalar2=mh_bc[:, 0:1], op0=ALU.mult, op1=ALU.add)
            nrm = small.tile([128, w], F32, name=f"nrm{c0}", tag="nrm")
            nc.scalar.activation(out=nrm[:], in_=ssq[:, c0:c1], func=AF.Sqrt)
            rin = small.tile([128, w], F32, name=f"rin{c0}", tag="rin")
            nc.vector.reciprocal(out=rin[:], in_=nrm[:])
            sim = small.tile([128, w], F32, name=f"sim{c0}", tag="sim")
            nc.vector.tensor_mul(sim[:], num[:, c0:c1], rin[:])
            nc.scalar.activation(out=u_bf[:, c0:c1], in_=sim[:], func=AF.Exp)

        def chunk_mm(vt_bf, b, c, r):
            nc.tensor.matmul(
                psum_r[0:1, :], u_bf[0:r, c:c + 1], vt_bf[0:r, b, :],
                start=(c == 0), stop=(c == nch - 1),
            )

        tiles = []
        c = 0
        sg_start = 0
        for g in range(ngrp):
            r0, r1 = g * 1024, (g + 1) * 1024
            kt = kpool.tile([128, 8, 256], F32, name=f"kt{g}", tag="kt")
            nc.sync.dma_start(out=kt[:], in_=keys[r0:r1, :].rearrange("(b p) h -> p b h", p=128))
            vt = vpool.tile([128, 8, 256], F32, name=f"vt{g}", tag="vt")
            nc.sync.dma_start(out=vt[:], in_=vals[r0:r1, :].rearrange("(b p) h -> p b h", p=128))
            vt_bf = vbfpool.tile([128, 8, 256], BF16, name=f"vb{g}", tag="vbf")
            nc.gpsimd.tensor_copy(vt_bf[:], vt[:])
            for b in range(8):
                chunk_sim(kt, b, c, 128)
                tiles.append((vt_bf, b, c, 128))
                c += 1
            if (g + 1) % 4 == 0:
                super_u(sg_start, c)
                for (vt_, b_, c_, r_) in tiles:
                    chunk_mm(vt_, b_, c_, r_)
                tiles = []
                sg_start = c

        nb_l = lf + (1 if rem else 0)
        if nb_l:
            ktl = kpool.tile([128, 8, 256], F32, name="ktl", tag="kt")
            vtl = vpool.tile([128, 8, 256], F32, name="vtl", tag="vt")
            r0 = ngrp * 1024
            if lf:
                nc.sync.dma_start(
                    out=ktl[:, 0:lf, :],
                    in_=keys[r0:r0 + lf * 128, :].rearrange("(b p) h -> p b h", p=128))
                nc.sync.dma_start(
                    out=vtl[:, 0:lf, :],
                    in_=vals[r0:r0 + lf * 128, :].rearrange("(b p) h -> p b h", p=128))
            if rem:
                nc.sync.dma_start(out=ktl[0:rem, lf, :], in_=keys[n_full * 128:, :])
                nc.sync.dma_start(out=vtl[0:rem, lf, :], in_=vals[n_full * 128:, :])
            vtl_bf = vbfpool.tile([128, 8, 256], BF16, name="vbl", tag="vbf")
            if lf:
                nc.gpsimd.tensor_copy(vtl_bf[:, 0:lf, :], vtl[:, 0:lf, :])
            if rem:
                nc.gpsimd.tensor_copy(vtl_bf[0:rem, lf, :], vtl[0:rem, lf, :])
            for b in range(nb_l):
                r = 128 if b < lf else rem
                chunk_sim(ktl, b, c, r)
                tiles.append((vtl_bf, b, c, r))
                c += 1
        if c > sg_start:
            super_u(sg_start, c)
            for (vt_, b_, c_, r_) in tiles:
                chunk_mm(vt_, b_, c_, r_)

    # ---- local partition function s = sum(u) ----
    s_col = T([128, 1], "s_col")
    nc.vector.reduce_sum(s_col[:], u_bf[:, :], axis=AX.X)
    with tc.tile_pool(name="pssm", bufs=1, space="PSUM") as pssm:
        psum_s = pssm.tile([128, 256], F32, name="psum_s", tag="ps")
        nc.tensor.matmul(psum_s[0:1, 0:1], ones_col[:], s_col[:])

        # ---- stage + AllReduce (r[256] | s | pad -> 264 floats) ----
        stage_r = T([1, 256], "stage_r")
        nc.vector.tensor_copy(stage_r[:], psum_r[0:1, :])
        stage_s = T([1, 8], "stage_s")
        nc.vector.memset(stage_s[:], 0.0)
        nc.vector.tensor_copy(stage_s[0:1, 0:1], psum_s[0:1, 0:1])

        cc_in = T([264], "cc_in", space="DRAM")
        cc_out = T([264], "cc_out", space="DRAM",
                   addr_space="Shared" if n_cores > 4 else "Local")
        nc.sync.dma_start(out=cc_in[0:256], in_=stage_r[0:1, :])
        nc.sync.dma_start(out=cc_in[256:264], in_=stage_s[0:1, :])
        nc.gpsimd.collective_compute(
            "AllReduce", ALU.add,
            replica_groups=[list(range(n_cores))],
            ins=[cc_in[:]], outs=[cc_out[:]],
        )
        rb = T([128, 2], "rb")
        nc.sync.dma_start(out=rb[:], in_=cc_out[0:256].rearrange("(j p) -> p j", p=128))
        sg = T([1, 1], "sg")
        nc.sync.dma_start(out=sg[:], in_=cc_out[256:257])

        # ---- m = tanh(r / s) ----
        psum_sb = pssm.tile([128, 256], F32, name="psum_sb", tag="ps")
        nc.tensor.matmul(psum_sb[0:128, 0:1], ones_row[:], sg[:])
        s_bc = T([128, 1], "s_bc")
        nc.vector.tensor_copy(s_bc[:], psum_sb[0:128, 0:1])
        s_inv = T([128, 1], "s_inv")
        nc.vector.reciprocal(out=s_inv[:], in_=s_bc[:])
        m_cols = T([128, 2], "m_cols")
        nc.scalar.activation(out=m_cols[:], in_=rb[:], func=AF.Tanh, scale=s_inv[:, 0:1])

        # ---- cell update ----
        t1 = T([128, 2], "t1")
        nc.vector.tensor_mul(t1[:], gsb[:, 0:2], cprev[:])
        t2 = T([128, 2], "t2")
        nc.vector.tensor_mul(t2[:], gsb[:, 2:4], cnsb[:])
        ct_a = T([128, 2], "ct_a")
        nc.vector.tensor_add(ct_a[:], t1[:], t2[:])
        t3 = T([128, 2], "t3")
        nc.vector.tensor_mul(t3[:], gsb[:, 6:8], m_cols[:])
        ct = T([128, 2], "ct")
        nc.vector.tensor_add(ct[:], ct_a[:], t3[:])
        tct = T([128, 2], "tct")
        nc.scalar.activation(out=tct[:], in_=ct[:], func=AF.Tanh)
        ht = T([128, 2], "ht")
        nc.vector.tensor_mul(ht[:], gsb[:, 4:6], tct[:])

        nc.sync.dma_start(
            out=out_vec[0:1, 0:256].rearrange("one (j p) -> one p j", p=128), in_=ht[:])
        nc.sync.dma_start(
            out=out_vec[0:1, 256:512].rearrange("one (j p) -> one p j", p=128), in_=ct[:])

        # ---- A2C head ----
        hrow_t = T([1, 256], "hrow_t")
        nc.sync.dma_start(out=hrow_t[0:1, 0:128], in_=ht[:, 0:1])
        nc.sync.dma_start(out=hrow_t[0:1, 128:256], in_=ht[:, 1:2])
        psum_h8 = pssm.tile([128, 256], F32, name="psum_h8", tag="ps")
        nc.tensor.matmul(psum_h8[0:8, 0:256], ones_row[0:1, 0:8], hrow_t[:])
        h8_sb = T([8, 256], "h8_sb")
        nc.vector.tensor_copy(h8_sb[:], psum_h8[0:8, 0:256])
        scr_l = T([8, 256], "scr_l")
        lg0 = T([8, 1], "lg0")
        nc.vector.tensor_mul(scr_l[:], Wa_sb[:], h8_sb[:])
        nc.scalar.activation(out=scr_l[:], in_=scr_l[:], func=AF.Copy,
                             accum_out=lg0[:])
        lg = T([8, 1], "lg")
        nc.vector.tensor_add(lg[:], lg0[:], ba_sb[:])
        lrow = T([1, 8], "lrow")
        nc.sync.dma_start(out=lrow[0:1, 0:8], in_=lg[:])
        e_row = T([1, 8], "e_row")
        nc.scalar.activation(out=e_row[:], in_=lrow[:], func=AF.Exp)
        esum = T([1, 1], "esum")
        nc.vector.reduce_sum(esum[:], e_row[:], axis=AX.X)
        einv = T([1, 1], "einv")
        nc.vector.reciprocal(out=einv[:], in_=esum[:])
        pi_row = T([1, 8], "pi_row")
        nc.vector.tensor_scalar_mul(pi_row[:], e_row[:], einv[0:1, 0:1])

        scr_v = T([1, 256], "scr_v")
        v0 = T([1, 1], "v0")
        nc.vector.tensor_mul(scr_v[:], Wc_sb[:], hrow_t[:])
        nc.scalar.activation(out=scr_v[:], in_=scr_v[:], func=AF.Copy,
                             accum_out=v0[:])
        v_sc = T([1, 1], "v_sc")
        nc.vector.tensor_add(v_sc[:], v0[:], bc_sb[0:1, 0:1])
        nc.sync.dma_start(out=out_vec[0:1, 512:520], in_=pi_row[0:1, :])
        nc.sync.dma_start(out=out_vec[0:1, 520:521], in_=v_sc[0:1, :])


def build(shard_rows=SHARD, n_cores=N_CORES):
    nc = bacc.Bacc(
        "TRN2",
        target_bir_lowering=False,
        debug=False,
        enable_asserts=False,
        num_devices=n_cores,
    )
    t = {}
    t["keys"] = nc.dram_tensor("keys", [shard_rows, 256], F32, kind="ExternalInput")
    t["vals"] = nc.dram_tensor("vals", [shard_rows, 256], F32, kind="ExternalInput")
    t["x_in"] = nc.dram_tensor("x_in", [256], F32, kind="ExternalInput")
    t["h_in"] = nc.dram_tensor("h_in", [256], F32, kind="ExternalInput")
    t["c_in"] = nc.dram_tensor("c_in", [256], F32, kind="ExternalInput")
    t["Wi_in"] = nc.dram_tensor("Wi_in", [1280, 256], F32, kind="ExternalInput")
    t["Wh_in"] = nc.dram_tensor("Wh_in", [1280, 256], F32, kind="ExternalInput")
    t["bias_in"] = nc.dram_tensor("bias_in", [1280], F32, kind="ExternalInput")
    t["Wa_in"] = nc.dram_tensor("Wa_in", [8, 256], F32, kind="ExternalInput")
    t["ba_in"] = nc.dram_tensor("ba_in", [8], F32, kind="ExternalInput")
    t["Wc_in"] = nc.dram_tensor("Wc_in", [1, 256], F32, kind="ExternalInput")
    t["bc_in"] = nc.dram_tensor("bc_in", [1], F32, kind="ExternalInput")
    t["out_vec"] = nc.dram_tensor("out_vec", [1, OUTW], F32, kind="ExternalOutput")

    with tile.TileContext(nc) as tc:
        _body(nc, tc, shard_rows, n_cores, t)
    nc.compile()
    return nc


def make_in_maps(x_t, h, c, keys, vals, Wi, bi, Wh, bh, Wa, ba, Wc, bc,
                 shard_rows=SHARD, n_cores=N_CORES):
    f = lambda a: np.ascontiguousarray(np.asarray(a, dtype=np.float32))
    keys = f(keys)
    vals = f(vals)
    common = {
        "x_in": f(x_t).reshape(256),
        "h_in": f(h).reshape(256),
        "c_in": f(c).reshape(256),
        "Wi_in": f(Wi),
        "Wh_in": f(Wh),
        "bias_in": f(bi) + f(bh),
        "Wa_in": f(Wa),
        "ba_in": f(ba),
        "Wc_in": f(Wc).reshape(1, 256),
        "bc_in": f(bc).reshape(1),
    }
    in_maps = []
    for i in range(n_cores):
        m = dict(common)
        m["keys"] = np.ascontiguousarray(keys[i * shard_rows:(i + 1) * shard_rows])
        m["vals"] = np.ascontiguousarray(vals[i * shard_rows:(i + 1) * shard_rows])
        in_maps.append(m)
    return in_maps


_NC_CACHE = None


def kernel(x_t, h, c, keys, vals, Wi, bi, Wh, bh, Wa, ba, Wc, bc, write_idx,
           _trace=False):
    global _NC_CACHE
    if _NC_CACHE is None:
        _NC_CACHE = build()
    nc = _NC_CACHE

    in_maps = make_in_maps(x_t, h, c, keys, vals, Wi, bi, Wh, bh, Wa, ba, Wc, bc)
    res = run_bass_kernel_spmd(nc, in_maps, core_ids=list(range(N_CORES)),
                               trace=_trace)
    out_vec = np.asarray(res.results[0]["out_vec"], dtype=np.float32)

    new_keys = np.array(np.asarray(keys, dtype=np.float32), copy=True)
    new_vals = np.array(np.asarray(vals, dtype=np.float32), copy=True)
    wi = int(write_idx)
    new_keys[wi] = out_vec[0, :H]
    new_vals[wi] = out_vec[0, H:2 * H]

    if _trace:
        kernel.last_exec_time_ns = res.exec_time_ns
        kernel.last_results = res
    return out_vec, new_keys, new_vals


# revision 17
# speedup vs baseline: 1.4457x; 1.4457x over previous
"""DND-LSTM cell, distributed over 8 TRN2 NeuronCores.

Sharding: the episodic dictionary (keys/vals, [100000, 256]) is split into 8
row-shards of 12500. Each core streams its keys shard from HBM, computes
unnormalized cosine-softmax weights u_i = exp((k_i . qhat)/||k_i||), a local
partial retrieval r_loc = sum_i u_i * vals_i (PE matmuls into PSUM) and the
local partition function s_loc = sum_i u_i. One tiny (264-float) AllReduce
produces the global (r, s); every core then finishes the LSTM cell + A2C head
redundantly. The LSTM gate matvecs (Wi/Wh) are computed on-device, overlapped
with the keys/vals streaming.

The new_keys/new_vals outputs equal the inputs except for one row
(write_idx), whose new content (h_t, c_t) is part of out_vec; the row update
is applied host-side after the gather.
"""

import numpy as np

import concourse.bass as bass
import concourse.bacc as bacc
import concourse.mybir as mybir
import concourse.tile as tile
from concourse.bass_utils import run_bass_kernel_spmd

F32 = mybir.dt.float32
BF16 = mybir.dt.bfloat16
AF = mybir.ActivationFunctionType
ALU = mybir.AluOpType
AX = mybir.AxisListType

H = 256
NG = 4
OUT = 8
DICT = 100000
N_CORES = 8
SHARD = DICT // N_CORES  # 12500
OUTW = 2 * H + OUT + 1  # 521


def _body(nc, tc, shard_rows, n_cores, t):
    persist = tc.alloc_tile_pool(name="persist", bufs=1)
    persistB = tc.alloc_tile_pool(name="persistB", bufs=1)
    persist_psum = tc.alloc_tile_pool(name="persist_psum", bufs=1, space="PSUM")
    persist_dram = tc.alloc_tile_pool(name="persist_dram", bufs=1, space="DRAM")

    def T(shape, name, space="SBUF", addr_space="Local"):
        pool = {"SBUF": persist, "PSUM": persist_psum, "DRAM": persist_dram}[space]
        return pool.tile(shape, F32, name=name, tag=name, addr_space=addr_space)

    try:
        _body_inner(nc, tc, shard_rows, n_cores, t, T, persistB)
    finally:
        persist_dram.release()
        persist_psum.release()
        persistB.release()
        persist.release()


def _body_inner(nc, tc, shard_rows, n_cores, t, T, persistB):

    n_full = shard_rows // 128
    rem = shard_rows % 128
    nch = n_full + (1 if rem else 0)
    ngrp = n_full // 8
    lf = n_full % 8

    keys, vals = t["keys"].ap(), t["vals"].ap()
    x_in, h_in, c_in = t["x_in"].ap(), t["h_in"].ap(), t["c_in"].ap()
    Wi_in, Wh_in, bias_in = t["Wi_in"].ap(), t["Wh_in"].ap(), t["bias_in"].ap()
    Wa_in, ba_in = t["Wa_in"].ap(), t["ba_in"].ap()
    Wc_in, bc_in = t["Wc_in"].ap(), t["bc_in"].ap()
    out_vec = t["out_vec"].ap()

    # ---- constants / small inputs ----
    ones_col = T([128, 1], "ones_col")
    nc.vector.memset(ones_col[:], 1.0)

    x_row = T([1, 256], "x_row")
    nc.sync.dma_start(out=x_row[:], in_=x_in[:])
    h_row0 = T([1, 256], "h_row0")
    nc.sync.dma_start(out=h_row0[:], in_=h_in[:])
    cprev = T([128, 2], "cprev")
    nc.sync.dma_start(out=cprev[:], in_=c_in[:].rearrange("(j p) -> p j", p=128))

    biasc = T([128, 10], "biasc")
    nc.sync.dma_start(out=biasc[:], in_=bias_in[:].rearrange("(t p) -> p t", p=128))
    ba_row = T([1, 8], "ba_row")
    nc.sync.dma_start(out=ba_row[:], in_=ba_in[:])
    bc_sb = T([1, 1], "bc_sb")
    nc.sync.dma_start(out=bc_sb[:], in_=bc_in[:])
    # transposed A2C head weights [d-chunk partitions, 8 pi cols + 1 v col]
    wat0 = T([128, 9], "wat0")
    wat1 = T([128, 9], "wat1")
    nc.sync.dma_start(out=wat0[:, 0:8], in_=Wa_in[:, 0:128].rearrange("a p -> p a"))
    nc.sync.dma_start(out=wat1[:, 0:8], in_=Wa_in[:, 128:256].rearrange("a p -> p a"))
    nc.sync.dma_start(out=wat0[:, 8:9], in_=Wc_in[:, 0:128].rearrange("o p -> p o"))
    nc.sync.dma_start(out=wat1[:, 8:9], in_=Wc_in[:, 128:256].rearrange("o p -> p o"))

    # ---- query normalization: qhat = h / ||h|| ----
    scr_h = T([1, 256], "scr_h")
    hh = T([1, 1], "hh")
    nc.scalar.activation(out=scr_h[:], in_=h_row0[:], func=AF.Square,
                         accum_out=hh[:])
    hnorm = T([1, 1], "hnorm")
    nc.scalar.activation(out=hnorm[:], in_=hh[:], func=AF.Sqrt)
    hinv = T([1, 1], "hinv")
    nc.vector.reciprocal(out=hinv[:], in_=hnorm[:])
    qn_row = T([1, 256], "qn_row")
    nc.vector.tensor_scalar_mul(qn_row[:], h_row0[:], hinv[0:1, 0:1])

    # ---- broadcasts to 128 partitions (DMA partition-broadcast) ----
    qn_b = T([128, 256], "qn_b")
    nc.gpsimd.partition_broadcast(qn_b[:], qn_row[0:1, :])
    x_b = T([128, 256], "x_b")
    nc.gpsimd.partition_broadcast(x_b[:], x_row[0:1, :])
    h_b = T([128, 256], "h_b")
    nc.gpsimd.partition_broadcast(h_b[:], h_row0[0:1, :])

    # ---- LSTM gate pre-activations (overlap the main stream) ----
    pre1 = T([128, 10], "pre1")
    pre2 = T([128, 10], "pre2")
    with tc.tile_pool(name="scrw", bufs=2) as scrw:
        Wi_sb = scrw.tile([128, 10, 256], F32, name="Wi_sb", tag="W_sb")
        nc.sync.dma_start(out=Wi_sb[:], in_=Wi_in[:, :].rearrange("(t p) h -> p t h", p=128))
        Wh_sb = scrw.tile([128, 10, 256], F32, name="Wh_sb", tag="W_sb")
        nc.sync.dma_start(out=Wh_sb[:], in_=Wh_in[:, :].rearrange("(t p) h -> p t h", p=128))
        sw1 = scrw.tile([128, 10, 256], F32, name="sw1", tag="scrw")
        nc.vector.tensor_mul(sw1[:], Wi_sb[:], x_b[:].rearrange("p (one h) -> p one h", one=1).broadcast_to([128, 10, 256]))
        nc.vector.reduce_sum(pre1[:], sw1[:], axis=AX.X)
        sw2 = scrw.tile([128, 10, 256], F32, name="sw2", tag="scrw")
        nc.vector.tensor_mul(sw2[:], Wh_sb[:], h_b[:].rearrange("p (one h) -> p one h", one=1).broadcast_to([128, 10, 256]))
        nc.vector.reduce_sum(pre2[:], sw2[:], axis=AX.X)
    pre_s = T([128, 10], "pre_s")
    nc.vector.tensor_add(pre_s[:], pre1[:], pre2[:])
    pre_b = T([128, 10], "pre_b")
    nc.vector.tensor_add(pre_b[:], pre_s[:], biasc[:])
    gsb = T([128, 8], "gsb")
    nc.scalar.activation(out=gsb[:], in_=pre_b[:, 0:8], func=AF.Sigmoid)
    cnsb = T([128, 2], "cnsb")
    nc.scalar.activation(out=cnsb[:], in_=pre_b[:, 8:10], func=AF.Tanh)
    # f*c_prev + i*c_new, ready before the collective lands
    ct_partial = T([128, 2], "ct_partial")
    tpa = T([128, 2], "tpa")
    nc.vector.tensor_mul(tpa[:], gsb[:, 0:2], cprev[:])
    tpb = T([128, 2], "tpb")
    nc.vector.tensor_mul(tpb[:], gsb[:, 2:4], cnsb[:])
    nc.vector.tensor_add(ct_partial[:], tpa[:], tpb[:])

    # ---- main stream over the dictionary shard ----
    # full groups use a p-major layout: partition p holds 8 consecutive rows
    # (8 KB contiguous per partition -> large DMA packets). Row order inside
    # num/u is permuted vs DRAM, which is harmless: r and s are row-order
    # invariant as long as keys and vals use the same mapping.
    num = T([128, nch], "num")
    ssq = T([128, nch], "ssq")
    u = T([128, nch], "u")
    # pad lanes of the partial last chunk keep num=-100, ssq=1 ->
    # u = exp(-100) == 0, contributing nothing to r or s
    nc.vector.memset(num[:], -100.0)
    nc.vector.memset(ssq[:], 1.0)

    psum_r = T([1, 256], "psum_r", space="PSUM")

    # supergroup boundaries (in groups): last ones smaller to cut PE drain
    sg_bounds = []
    gacc = 0
    while gacc + 4 <= ngrp - 3:
        gacc += 4
        sg_bounds.append(gacc)
    # remaining full groups in pairs
    while gacc + 2 <= ngrp:
        gacc += 2
        sg_bounds.append(gacc)
    if gacc < ngrp:
        sg_bounds.append(ngrp)
    dve_ssq_groups = set(g for g in range(ngrp) if g % 6 == 5)

    with (
        tc.tile_pool(name="kt", bufs=5) as kpool,
        tc.tile_pool(name="vt", bufs=6) as vpool,
        tc.tile_pool(name="scrg", bufs=3) as scrg,
        tc.tile_pool(name="small", bufs=3) as small,
        tc.tile_pool(name="scra", bufs=4) as scra,
    ):
        def group_sim(kt, g):
            c0 = g * 8
            sk = scrg.tile([128, 8, 256], F32, name=f"sk{g}", tag="scrg")
            nc.vector.tensor_mul(sk[:], kt[:], qn_b[:].rearrange("p (one h) -> p one h", one=1).broadcast_to([128, 8, 256]))
            nc.vector.reduce_sum(num[:, c0:c0 + 8], sk[:], axis=AX.X)
            if g in dve_ssq_groups:
                sq = scrg.tile([128, 8, 256], F32, name=f"sq{g}", tag="scrg")
                nc.vector.tensor_mul(sq[:], kt[:], kt[:])
                nc.vector.reduce_sum(ssq[:, c0:c0 + 8], sq[:], axis=AX.X)
            else:
                for b in range(8):
                    sa = scra.tile([128, 256], F32, name=f"sa{g}_{b}", tag="scra")
                    nc.scalar.activation(
                        out=sa[:], in_=kt[:, b, :], func=AF.Square,
                        accum_out=ssq[:, c0 + b:c0 + b + 1])

        def chunk_sim(kt, b, c, r):
            sk = scra.tile([128, 256], F32, name=f"skc{c}", tag="scra")
            nc.vector.tensor_mul(sk[0:r, :], kt[0:r, b, :], qn_b[0:r, :])
            nc.vector.reduce_sum(num[0:r, c:c + 1], sk[0:r, :], axis=AX.X)
            sb_ = scra.tile([128, 256], F32, name=f"sbc{c}", tag="scra")
            nc.scalar.activation(out=sb_[0:r, :], in_=kt[0:r, b, :],
                                 func=AF.Square, accum_out=ssq[0:r, c:c + 1])

        def super_u(c0, c1):
            w = c1 - c0
            nrm = small.tile([128, w], F32, name=f"nrm{c0}", tag="nrm")
            nc.scalar.activation(out=nrm[:], in_=ssq[:, c0:c1], func=AF.Sqrt)
            rin = small.tile([128, w], F32, name=f"rin{c0}", tag="rin")
            nc.vector.reciprocal(out=rin[:], in_=nrm[:])
            sim = small.tile([128, w], F32, name=f"sim{c0}", tag="sim")
            nc.vector.tensor_mul(sim[:], num[:, c0:c1], rin[:])
            nc.scalar.activation(out=u[:, c0:c1], in_=sim[:], func=AF.Exp)

        def chunk_mm(vt, b, c, r):
            nc.tensor.matmul(
                psum_r[0:1, :], u[0:r, c:c + 1], vt[0:r, b, :],
                start=(c == 0), stop=(c == nch - 1),
            )

        tiles = []
        c = 0
        sg_start = 0
        done_g = 0
        for sg_end in sg_bounds:
            for g in range(done_g, sg_end):
                r0, r1 = g * 1024, (g + 1) * 1024
                kt = kpool.tile([128, 8, 256], F32, name=f"kt{g}", tag="kt")
                nc.sync.dma_start(out=kt[:], in_=keys[r0:r1, :].rearrange("(p b) h -> p b h", b=8))
                vt = vpool.tile([128, 8, 256], F32, name=f"vt{g}", tag="vt")
                nc.sync.dma_start(out=vt[:], in_=vals[r0:r1, :].rearrange("(p b) h -> p b h", b=8))
                group_sim(kt, g)
                for b in range(8):
                    tiles.append((vt, b, c, 128))
                    c += 1
            done_g = sg_end
            super_u(sg_start, c)
            for (vt_, b_, c_, r_) in tiles:
                chunk_mm(vt_, b_, c_, r_)
            tiles = []
            sg_start = c

        # leftover rows (row-per-partition layout, per-chunk ops)
        nb_l = lf + (1 if rem else 0)
        if nb_l:
            ktl = kpool.tile([128, 8, 256], F32, name="ktl", tag="kt")
            vtl = vpool.tile([128, 8, 256], F32, name="vtl", tag="vt")
            r0 = ngrp * 1024
            if lf:
                nc.sync.dma_start(
                    out=ktl[:, 0:lf, :],
                    in_=keys[r0:r0 + lf * 128, :].rearrange("(b p) h -> p b h", p=128))
                nc.sync.dma_start(
                    out=vtl[:, 0:lf, :],
                    in_=vals[r0:r0 + lf * 128, :].rearrange("(b p) h -> p b h", p=128))
            if rem:
                nc.sync.dma_start(out=ktl[0:rem, lf, :], in_=keys[n_full * 128:, :])
                nc.sync.dma_start(out=vtl[0:rem, lf, :], in_=vals[n_full * 128:, :])
            for b in range(nb_l):
                r = 128 if b < lf else rem
                chunk_sim(ktl, b, c, r)
                tiles.append((vtl, b, c, r))
                c += 1
            super_u(sg_start, c)
            for (vt_, b_, c_, r_) in tiles:
                chunk_mm(vt_, b_, c_, r_)

    # ---- local partition function s = sum(u) ----
    s_col = T([128, 1], "s_col")
    nc.vector.reduce_sum(s_col[:], u[:, :], axis=AX.X)
    with tc.tile_pool(name="pssm", bufs=1, space="PSUM") as pssm:
        psum_s = pssm.tile([128, 256], F32, name="psum_s", tag="ps")
        nc.tensor.matmul(psum_s[0:1, 0:1], ones_col[:], s_col[:])

        # ---- stage + AllReduce (r[256] | s | pad -> 264 floats) ----
        stage_r = T([1, 256], "stage_r")
        nc.vector.tensor_copy(stage_r[:], psum_r[0:1, :])
        stage_s = T([1, 8], "stage_s")
        nc.vector.memset(stage_s[:], 0.0)
        nc.vector.tensor_copy(stage_s[0:1, 0:1], psum_s[0:1, 0:1])

        cc_in = T([264], "cc_in", space="DRAM")
        cc_out = T([264], "cc_out", space="DRAM",
                   addr_space="Shared" if n_cores > 4 else "Local")
        nc.sync.dma_start(out=cc_in[0:256], in_=stage_r[0:1, :])
        nc.sync.dma_start(out=cc_in[256:264], in_=stage_s[0:1, :])
        nc.gpsimd.collective_compute(
            "AllReduce", ALU.add,
            replica_groups=[list(range(n_cores))],
            ins=[cc_in[:]], outs=[cc_out[:]],
        )
        rb = T([128, 2], "rb")
        nc.sync.dma_start(out=rb[:], in_=cc_out[0:256].rearrange("(j p) -> p j", p=128))
        sg_sb = T([1, 1], "sg_sb")
        nc.sync.dma_start(out=sg_sb[:], in_=cc_out[256:257])
        s_bc = T([128, 1], "s_bc")
        nc.gpsimd.partition_broadcast(s_bc[:], sg_sb[0:1, :])

        # ---- m = tanh(r / s); cell update ----
        s_inv = T([128, 1], "s_inv")
        nc.vector.reciprocal(out=s_inv[:], in_=s_bc[:])
        m_cols = T([128, 2], "m_cols")
        nc.scalar.activation(out=m_cols[:], in_=rb[:], func=AF.Tanh, scale=s_inv[:, 0:1])
        t3 = T([128, 2], "t3")
        nc.vector.tensor_mul(t3[:], gsb[:, 6:8], m_cols[:])
        ct = T([128, 2], "ct")
        nc.vector.tensor_add(ct[:], ct_partial[:], t3[:])
        tct = T([128, 2], "tct")
        nc.scalar.activation(out=tct[:], in_=ct[:], func=AF.Tanh)
        ht = T([128, 2], "ht")
        nc.vector.tensor_mul(ht[:], gsb[:, 4:6], tct[:])

        nc.sync.dma_start(
            out=out_vec[0:1, 0:256].rearrange("one (j p) -> one p j", p=128), in_=ht[:])
        nc.sync.dma_start(
            out=out_vec[0:1, 256:512].rearrange("one (j p) -> one p j", p=128), in_=ct[:])

        # ---- A2C head via transposed weights: psum_hd = [logits | v] ----
        psum_hd = pssm.tile([128, 256], F32, name="psum_hd", tag="ps")
        nc.tensor.matmul(psum_hd[0:1, 0:9], ht[:, 0:1], wat0[:], start=True, stop=False)
        nc.tensor.matmul(psum_hd[0:1, 0:9], ht[:, 1:2], wat1[:], start=False, stop=True)
        hd = T([1, 9], "hd")
        nc.vector.tensor_copy(hd[:], psum_hd[0:1, 0:9])
        lg_row = T([1, 8], "lg_row")
        nc.vector.tensor_add(lg_row[:], hd[0:1, 0:8], ba_row[:])
        e_row = T([1, 8], "e_row")
        nc.scalar.activation(out=e_row[:], in_=lg_row[:], func=AF.Exp)
        esum = T([1, 1], "esum")
        nc.vector.reduce_sum(esum[:], e_row[:], axis=AX.X)
        einv = T([1, 1], "einv")
        nc.vector.reciprocal(out=einv[:], in_=esum[:])
        pv = T([1, 9], "pv")
        nc.vector.tensor_scalar_mul(pv[0:1, 0:8], e_row[:], einv[0:1, 0:1])
        nc.vector.tensor_add(pv[0:1, 8:9], hd[0:1, 8:9], bc_sb[0:1, 0:1])
        nc.sync.dma_start(out=out_vec[0:1, 512:521], in_=pv[0:1, :])



def build(shard_rows=SHARD, n_cores=N_CORES):
    nc = bacc.Bacc(
        "TRN2",
        target_bir_lowering=False,
        debug=False,
        enable_asserts=False,
        num_devices=n_cores,
    )
    t = {}
    t["keys"] = nc.dram_tensor("keys", [shard_rows, 256], F32, kind="ExternalInput")
    t["vals"] = nc.dram_tensor("vals", [shard_rows, 256], F32, kind="ExternalInput")
    t["x_in"] = nc.dram_tensor("x_in", [256], F32, kind="ExternalInput")
    t["h_in"] = nc.dram_tensor("h_in", [256], F32, kind="ExternalInput")
    t["c_in"] = nc.dram_tensor("c_in", [256], F32, kind="ExternalInput")
    t["Wi_in"] = nc.dram_tensor("Wi_in", [1280, 256], F32, kind="ExternalInput")
    t["Wh_in"] = nc.dram_tensor("Wh_in", [1280, 256], F32, kind="ExternalInput")
    t["bias_in"] = nc.dram_tensor("bias_in", [1280], F32, kind="ExternalInput")
    t["Wa_in"] = nc.dram_tensor("Wa_in", [8, 256], F32, kind="ExternalInput")
    t["ba_in"] = nc.dram_tensor("ba_in", [8], F32, kind="ExternalInput")
    t["Wc_in"] = nc.dram_tensor("Wc_in", [1, 256], F32, kind="ExternalInput")
    t["bc_in"] = nc.dram_tensor("bc_in", [1], F32, kind="ExternalInput")
    t["out_vec"] = nc.dram_tensor("out_vec", [1, OUTW], F32, kind="ExternalOutput")

    with tile.TileContext(nc) as tc:
        _body(nc, tc, shard_rows, n_cores, t)
    nc.compile()
    return nc


def make_in_maps(x_t, h, c, keys, vals, Wi, bi, Wh, bh, Wa, ba, Wc, bc,
                 shard_rows=SHARD, n_cores=N_CORES):
    f = lambda a: np.ascontiguousarray(np.asarray(a, dtype=np.float32))
    keys = f(keys)
    vals = f(vals)
    common = {
        "x_in": f(x_t).reshape(256),
        "h_in": f(h).reshape(256),
        "c_in": f(c).reshape(256),
        "Wi_in": f(Wi),
        "Wh_in": f(Wh),
        "bias_in": f(bi) + f(bh),
        "Wa_in": f(Wa),
        "ba_in": f(ba),
        "Wc_in": f(Wc).reshape(1, 256),
        "bc_in": f(bc).reshape(1),
    }
    in_maps = []
    for i in range(n_cores):
        m = dict(common)
        m["keys"] = np.ascontiguousarray(keys[i * shard_rows:(i + 1) * shard_rows])
        m["vals"] = np.ascontiguousarray(vals[i * shard_rows:(i + 1) * shard_rows])
        in_maps.append(m)
    return in_maps


_NC_CACHE = None


def kernel(x_t, h, c, keys, vals, Wi, bi, Wh, bh, Wa, ba, Wc, bc, write_idx,
           _trace=False):
    global _NC_CACHE
    if _NC_CACHE is None:
        _NC_CACHE = build()
    nc = _NC_CACHE

    in_maps = make_in_maps(x_t, h, c, keys, vals, Wi, bi, Wh, bh, Wa, ba, Wc, bc)
    res = run_bass_kernel_spmd(nc, in_maps, core_ids=list(range(N_CORES)),
                               trace=_trace)
    out_vec = np.asarray(res.results[0]["out_vec"], dtype=np.float32)

    new_keys = np.array(np.asarray(keys, dtype=np.float32), copy=True)
    new_vals = np.array(np.asarray(vals, dtype=np.float32), copy=True)
    wi = int(write_idx)
    new_keys[wi] = out_vec[0, :H]
    new_vals[wi] = out_vec[0, H:2 * H]

    if _trace:
        kernel.last_exec_time_ns = res.exec_time_ns
        kernel.last_results = res
    return out_vec, new_keys, new_vals


# revision 21
# speedup vs baseline: 1.5222x; 1.0529x over previous
"""DND-LSTM cell, distributed over 8 TRN2 NeuronCores.

Sharding: the episodic dictionary (keys/vals, [100000, 256]) is split into 8
row-shards of 12500. Each core streams its keys shard from HBM, computes
unnormalized cosine-softmax weights u_i = exp((k_i . qhat)/||k_i||), a local
partial retrieval r_loc = sum_i u_i * vals_i (PE matmuls into PSUM) and the
local partition function s_loc = sum_i u_i. One tiny (264-float) AllReduce
produces the global (r, s); every core then finishes the LSTM cell + A2C head
redundantly. The LSTM gate matvecs (Wi/Wh) are computed on-device, overlapped
with the keys/vals streaming.

The new_keys/new_vals outputs equal the inputs except for one row
(write_idx), whose new content (h_t, c_t) is part of out_vec; the row update
is applied host-side after the gather.
"""

import numpy as np

import concourse.bass as bass
import concourse.bacc as bacc
import concourse.mybir as mybir
import concourse.tile as tile
from concourse.bass_utils import run_bass_kernel_spmd

F32 = mybir.dt.float32
BF16 = mybir.dt.bfloat16
AF = mybir.ActivationFunctionType
ALU = mybir.AluOpType
AX = mybir.AxisListType

H = 256
NG = 4
OUT = 8
DICT = 100000
N_CORES = 8
SHARD = DICT // N_CORES  # 12500
OUTW = 2 * H + OUT + 1  # 521


def _body(nc, tc, shard_rows, n_cores, t):
    persist = tc.alloc_tile_pool(name="persist", bufs=1)
    persistB = tc.alloc_tile_pool(name="persistB", bufs=1)
    persist_psum = tc.alloc_tile_pool(name="persist_psum", bufs=1, space="PSUM")
    persist_dram = tc.alloc_tile_pool(name="persist_dram", bufs=1, space="DRAM")

    def T(shape, name, space="SBUF", addr_space="Local"):
        pool = {"SBUF": persist, "PSUM": persist_psum, "DRAM": persist_dram}[space]
        return pool.tile(shape, F32, name=name, tag=name, addr_space=addr_space)

    try:
        _body_inner(nc, tc, shard_rows, n_cores, t, T, persistB)
    finally:
        persist_dram.release()
        persist_psum.release()
        persistB.release()
        persist.release()


def _body_inner(nc, tc, shard_rows, n_cores, t, T, persistB):

    n_full = shard_rows // 128
    rem = shard_rows % 128
    nch = n_full + (1 if rem else 0)
    ngrp = n_full // 8
    lf = n_full % 8

    keys, vals = t["keys"].ap(), t["vals"].ap()
    x_in, h_in, c_in = t["x_in"].ap(), t["h_in"].ap(), t["c_in"].ap()
    Wi_in, Wh_in, bias_in = t["Wi_in"].ap(), t["Wh_in"].ap(), t["bias_in"].ap()
    Wa_in, ba_in = t["Wa_in"].ap(), t["ba_in"].ap()
    Wc_in, bc_in = t["Wc_in"].ap(), t["bc_in"].ap()
    out_vec = t["out_vec"].ap()

    # ---- constants / small inputs ----
    ones_col = T([128, 1], "ones_col")
    nc.vector.memset(ones_col[:], 1.0)

    x_row = T([1, 256], "x_row")
    nc.sync.dma_start(out=x_row[:], in_=x_in[:])
    h_row0 = T([1, 256], "h_row0")
    nc.sync.dma_start(out=h_row0[:], in_=h_in[:])
    cprev = T([128, 2], "cprev")
    nc.sync.dma_start(out=cprev[:], in_=c_in[:].rearrange("(j p) -> p j", p=128))

    biasc = T([128, 10], "biasc")
    nc.sync.dma_start(out=biasc[:], in_=bias_in[:].rearrange("(t p) -> p t", p=128))
    ba_row = T([1, 8], "ba_row")
    nc.sync.dma_start(out=ba_row[:], in_=ba_in[:])
    bc_sb = T([1, 1], "bc_sb")
    nc.sync.dma_start(out=bc_sb[:], in_=bc_in[:])
    # transposed A2C head weights [d-chunk partitions, 8 pi cols + 1 v col]
    wat0 = T([128, 9], "wat0")
    wat1 = T([128, 9], "wat1")
    nc.sync.dma_start(out=wat0[:, 0:8], in_=Wa_in[:, 0:128].rearrange("a p -> p a"))
    nc.sync.dma_start(out=wat1[:, 0:8], in_=Wa_in[:, 128:256].rearrange("a p -> p a"))
    nc.sync.dma_start(out=wat0[:, 8:9], in_=Wc_in[:, 0:128].rearrange("o p -> p o"))
    nc.sync.dma_start(out=wat1[:, 8:9], in_=Wc_in[:, 128:256].rearrange("o p -> p o"))

    # ---- query normalization: qhat = h / ||h|| ----
    scr_h = T([1, 256], "scr_h")
    hh = T([1, 1], "hh")
    nc.scalar.activation(out=scr_h[:], in_=h_row0[:], func=AF.Square,
                         accum_out=hh[:])
    hnorm = T([1, 1], "hnorm")
    nc.scalar.activation(out=hnorm[:], in_=hh[:], func=AF.Sqrt)
    hinv = T([1, 1], "hinv")
    nc.vector.reciprocal(out=hinv[:], in_=hnorm[:])
    qn_row = T([1, 256], "qn_row")
    nc.vector.tensor_scalar_mul(qn_row[:], h_row0[:], hinv[0:1, 0:1])

    # ---- broadcasts to 128 partitions (DMA partition-broadcast) ----
    qn_b = T([128, 256], "qn_b")
    nc.gpsimd.partition_broadcast(qn_b[:], qn_row[0:1, :])
    x_b = T([128, 256], "x_b")
    nc.gpsimd.partition_broadcast(x_b[:], x_row[0:1, :])
    h_b = T([128, 256], "h_b")
    nc.gpsimd.partition_broadcast(h_b[:], h_row0[0:1, :])

    # ---- LSTM gate pre-activations (overlap the main stream) ----
    pre1 = T([128, 10], "pre1")
    pre2 = T([128, 10], "pre2")
    with tc.tile_pool(name="scrw", bufs=2) as scrw:
        Wi_sb = scrw.tile([128, 10, 256], F32, name="Wi_sb", tag="W_sb")
        nc.sync.dma_start(out=Wi_sb[:], in_=Wi_in[:, :].rearrange("(t p) h -> p t h", p=128))
        Wh_sb = scrw.tile([128, 10, 256], F32, name="Wh_sb", tag="W_sb")
        nc.sync.dma_start(out=Wh_sb[:], in_=Wh_in[:, :].rearrange("(t p) h -> p t h", p=128))
        sw1 = scrw.tile([128, 10, 256], F32, name="sw1", tag="scrw")
        nc.vector.tensor_mul(sw1[:], Wi_sb[:], x_b[:].rearrange("p (one h) -> p one h", one=1).broadcast_to([128, 10, 256]))
        nc.vector.reduce_sum(pre1[:], sw1[:], axis=AX.X)
        sw2 = scrw.tile([128, 10, 256], F32, name="sw2", tag="scrw")
        nc.vector.tensor_mul(sw2[:], Wh_sb[:], h_b[:].rearrange("p (one h) -> p one h", one=1).broadcast_to([128, 10, 256]))
        nc.vector.reduce_sum(pre2[:], sw2[:], axis=AX.X)
    pre_s = T([128, 10], "pre_s")
    nc.vector.tensor_add(pre_s[:], pre1[:], pre2[:])
    pre_b = T([128, 10], "pre_b")
    nc.vector.tensor_add(pre_b[:], pre_s[:], biasc[:])
    gsb = T([128, 8], "gsb")
    nc.scalar.activation(out=gsb[:], in_=pre_b[:, 0:8], func=AF.Sigmoid)
    cnsb = T([128, 2], "cnsb")
    nc.scalar.activation(out=cnsb[:], in_=pre_b[:, 8:10], func=AF.Tanh)
    # f*c_prev + i*c_new, ready before the collective lands
    ct_partial = T([128, 2], "ct_partial")
    tpa = T([128, 2], "tpa")
    nc.vector.tensor_mul(tpa[:], gsb[:, 0:2], cprev[:])
    tpb = T([128, 2], "tpb")
    nc.vector.tensor_mul(tpb[:], gsb[:, 2:4], cnsb[:])
    nc.vector.tensor_add(ct_partial[:], tpa[:], tpb[:])

    # ---- main stream over the dictionary shard ----
    # full groups use a p-major layout: partition p holds 8 consecutive rows
    # (8 KB contiguous per partition -> large DMA packets). Row order inside
    # num/u is permuted vs DRAM, which is harmless: r and s are row-order
    # invariant as long as keys and vals use the same mapping.
    num = T([128, nch], "num")
    ssq = T([128, nch], "ssq")
    u = persistB.tile([128, nch], mybir.dt.float32r, name="u", tag="u")
    # pad lanes of the partial last chunk keep num=-100, ssq=1 ->
    # u = exp(-100) == 0, contributing nothing to r or s
    nc.vector.memset(num[:], -100.0)
    nc.vector.memset(ssq[:], 1.0)

    psum_r = T([1, 256], "psum_r", space="PSUM")

    # split the AllReduce: partial (r,s) over chunks [0, c_split) is reduced
    # across cores while the rest of the shard is still streaming
    c_split = 64 if nch > 80 else nch
    cc_in1 = T([264], "cc_in1", space="DRAM")
    cc_out1 = T([264], "cc_out1", space="DRAM",
                addr_space="Shared" if n_cores > 4 else "Local")
    cc_in2 = T([264], "cc_in2", space="DRAM")
    cc_out2 = T([264], "cc_out2", space="DRAM",
                addr_space="Shared" if n_cores > 4 else "Local")

    # supergroup boundaries (in groups): last ones smaller to cut PE drain
    sg_bounds = []
    gacc = 0
    while gacc + 4 <= ngrp - 3:
        gacc += 4
        sg_bounds.append(gacc)
    # remaining full groups in pairs
    while gacc + 2 <= ngrp:
        gacc += 2
        sg_bounds.append(gacc)
    if gacc < ngrp:
        sg_bounds.append(ngrp)
    dve_ssq_groups = set(g for g in range(ngrp) if g % 6 == 5)

    with (
        tc.tile_pool(name="kt", bufs=5) as kpool,
        tc.tile_pool(name="vt", bufs=6) as vpool,
        tc.tile_pool(name="scrg", bufs=3) as scrg,
        tc.tile_pool(name="small", bufs=3) as small,
        tc.tile_pool(name="scra", bufs=4) as scra,
    ):
        def group_sim(kt, g):
            c0 = g * 8
            sk = scrg.tile([128, 8, 256], F32, name=f"sk{g}", tag="scrg")
            nc.vector.tensor_mul(sk[:], kt[:], qn_b[:].rearrange("p (one h) -> p one h", one=1).broadcast_to([128, 8, 256]))
            nc.vector.reduce_sum(num[:, c0:c0 + 8], sk[:], axis=AX.X)
            if g in dve_ssq_groups:
                sq = scrg.tile([128, 8, 256], F32, name=f"sq{g}", tag="scrg")
                nc.vector.tensor_mul(sq[:], kt[:], kt[:])
                nc.vector.reduce_sum(ssq[:, c0:c0 + 8], sq[:], axis=AX.X)
            else:
                for b in range(8):
                    sa = scra.tile([128, 256], F32, name=f"sa{g}_{b}", tag="scra")
                    nc.scalar.activation(
                        out=sa[:], in_=kt[:, b, :], func=AF.Square,
                        accum_out=ssq[:, c0 + b:c0 + b + 1])

        def chunk_sim(kt, b, c, r):
            sk = scra.tile([128, 256], F32, name=f"skc{c}", tag="scra")
            nc.vector.tensor_mul(sk[0:r, :], kt[0:r, b, :], qn_b[0:r, :])
            nc.vector.reduce_sum(num[0:r, c:c + 1], sk[0:r, :], axis=AX.X)
            sb_ = scra.tile([128, 256], F32, name=f"sbc{c}", tag="scra")
            nc.scalar.activation(out=sb_[0:r, :], in_=kt[0:r, b, :],
                                 func=AF.Square, accum_out=ssq[0:r, c:c + 1])

        def super_u(c0, c1):
            w = c1 - c0
            nrm = small.tile([128, w], F32, name=f"nrm{c0}", tag="nrm")
            nc.scalar.activation(out=nrm[:], in_=ssq[:, c0:c1], func=AF.Sqrt)
            rin = small.tile([128, w], F32, name=f"rin{c0}", tag="rin")
            nc.vector.reciprocal(out=rin[:], in_=nrm[:])
            sim = small.tile([128, w], F32, name=f"sim{c0}", tag="sim")
            nc.vector.tensor_mul(sim[:], num[:, c0:c1], rin[:])
            nc.scalar.activation(out=u[:, c0:c1], in_=sim[:], func=AF.Exp)

        def chunk_mm(vt, b, c, r):
            nc.tensor.matmul(
                psum_r[0:1, :], u[0:r, c:c + 1], vt[0:r, b, :],
                start=(c == 0 or c == c_split), stop=(c == c_split - 1 or c == nch - 1),
            )

        ppart = tc.alloc_tile_pool(name="ppart", bufs=2, space="PSUM")
        stg = tc.alloc_tile_pool(name="stg", bufs=2)

        def fire_collective(ccin, ccout, c_lo, c_hi):
            scol = stg.tile([128, 1], F32, name=f"scol{c_lo}", tag="scol")
            nc.vector.reduce_sum(scol[:], u[:, c_lo:c_hi], axis=AX.X)
            ps = ppart.tile([128, 256], F32, name=f"ps{c_lo}", tag="pp")
            nc.tensor.matmul(ps[0:1, 0:1], ones_col[:], scol[:])
            stage = stg.tile([128, 264], F32, name=f"stage{c_lo}", tag="stage")
            nc.vector.memset(stage[0:1, :], 0.0)
            nc.vector.tensor_copy(stage[0:1, 0:256], psum_r[0:1, :])
            nc.vector.tensor_copy(stage[0:1, 256:257], ps[0:1, 0:1])
            nc.sync.dma_start(out=ccin[:], in_=stage[0:1, :])
            nc.gpsimd.collective_compute(
                "AllReduce", ALU.add,
                replica_groups=[list(range(n_cores))],
                ins=[ccin[:]], outs=[ccout[:]],
            )

        tiles = []
        c = 0
        sg_start = 0
        done_g = 0
        for sg_end in sg_bounds:
            for g in range(done_g, sg_end):
                r0, r1 = g * 1024, (g + 1) * 1024
                kt = kpool.tile([128, 8, 256], F32, name=f"kt{g}", tag="kt")
                nc.sync.dma_start(out=kt[:], in_=keys[r0:r1, :].rearrange("(p b) h -> p b h", b=8))
                vt = vpool.tile([128, 8, 256], mybir.dt.float32r, name=f"vt{g}", tag="vt")
                nc.gpsimd.dma_start(out=vt[:], in_=vals[r0:r1, :].rearrange("(p b) h -> p b h", b=8))
                group_sim(kt, g)
                for b in range(8):
                    tiles.append((vt, b, c, 128))
                    c += 1
            done_g = sg_end
            super_u(sg_start, c)
            for (vt_, b_, c_, r_) in tiles:
                chunk_mm(vt_, b_, c_, r_)
            tiles = []
            sg_start = c
            if c == c_split:
                fire_collective(cc_in1, cc_out1, 0, c_split)

        # leftover rows (row-per-partition layout, per-chunk ops)
        nb_l = lf + (1 if rem else 0)
        if nb_l:
            ktl = kpool.tile([128, 8, 256], F32, name="ktl", tag="kt")
            vtl = vpool.tile([128, 8, 256], mybir.dt.float32r, name="vtl", tag="vt")
            r0 = ngrp * 1024
            if lf:
                nc.sync.dma_start(
                    out=ktl[:, 0:lf, :],
                    in_=keys[r0:r0 + lf * 128, :].rearrange("(b p) h -> p b h", p=128))
                nc.gpsimd.dma_start(
                    out=vtl[:, 0:lf, :],
                    in_=vals[r0:r0 + lf * 128, :].rearrange("(b p) h -> p b h", p=128))
            if rem:
                nc.sync.dma_start(out=ktl[0:rem, lf, :], in_=keys[n_full * 128:, :])
                nc.gpsimd.dma_start(out=vtl[0:rem, lf, :], in_=vals[n_full * 128:, :])
            for b in range(nb_l):
                r = 128 if b < lf else rem
                chunk_sim(ktl, b, c, r)
                tiles.append((vtl, b, c, r))
                c += 1
            super_u(sg_start, c)
            for (vt_, b_, c_, r_) in tiles:
                chunk_mm(vt_, b_, c_, r_)

        if c_split < nch:
            fire_collective(cc_in2, cc_out2, c_split, nch)
        else:
            fire_collective(cc_in1, cc_out1, 0, nch)
        ppart.release()
        stg.release()

    with tc.tile_pool(name="pssm", bufs=1, space="PSUM") as pssm:
        # ---- combine the two partial reductions ----
        two = 1 if nch > c_split or True else 0
        rb1 = T([128, 2], "rb1")
        nc.sync.dma_start(out=rb1[:], in_=cc_out1[0:256].rearrange("(j p) -> p j", p=128))
        sg1 = T([1, 2], "sg1")
        nc.vector.memset(sg1[:], 0.0)
        nc.sync.dma_start(out=sg1[0:1, 0:1], in_=cc_out1[256:257])
        if c_split < nch:
            rb2 = T([128, 2], "rb2")
            nc.sync.dma_start(out=rb2[:], in_=cc_out2[0:256].rearrange("(j p) -> p j", p=128))
            rb = T([128, 2], "rb")
            nc.vector.tensor_add(rb[:], rb1[:], rb2[:])
            nc.sync.dma_start(out=sg1[0:1, 1:2], in_=cc_out2[256:257])
        else:
            rb = rb1
        sg_sb = T([1, 1], "sg_sb")
        nc.vector.reduce_sum(sg_sb[:], sg1[:], axis=AX.X)
        s_bc = T([128, 1], "s_bc")
        nc.gpsimd.partition_broadcast(s_bc[:], sg_sb[0:1, :])

        # ---- m = tanh(r / s); cell update ----
        s_inv = T([128, 1], "s_inv")
        nc.vector.reciprocal(out=s_inv[:], in_=s_bc[:])
        m_cols = T([128, 2], "m_cols")
        nc.scalar.activation(out=m_cols[:], in_=rb[:], func=AF.Tanh, scale=s_inv[:, 0:1])
        t3 = T([128, 2], "t3")
        nc.vector.tensor_mul(t3[:], gsb[:, 6:8], m_cols[:])
        ct = T([128, 2], "ct")
        nc.vector.tensor_add(ct[:], ct_partial[:], t3[:])
        tct = T([128, 2], "tct")
        nc.scalar.activation(out=tct[:], in_=ct[:], func=AF.Tanh)
        ht = T([128, 2], "ht")
        nc.vector.tensor_mul(ht[:], gsb[:, 4:6], tct[:])

        nc.sync.dma_start(
            out=out_vec[0:1, 0:256].rearrange("one (j p) -> one p j", p=128), in_=ht[:])
        nc.sync.dma_start(
            out=out_vec[0:1, 256:512].rearrange("one (j p) -> one p j", p=128), in_=ct[:])

        # ---- A2C head via transposed weights: psum_hd = [logits | v] ----
        psum_hd = pssm.tile([128, 256], F32, name="psum_hd", tag="ps")
        nc.tensor.matmul(psum_hd[0:1, 0:9], ht[:, 0:1], wat0[:], start=True, stop=False)
        nc.tensor.matmul(psum_hd[0:1, 0:9], ht[:, 1:2], wat1[:], start=False, stop=True)
        hd = T([1, 9], "hd")
        nc.vector.tensor_copy(hd[:], psum_hd[0:1, 0:9])
        lg_row = T([1, 8], "lg_row")
        nc.vector.tensor_add(lg_row[:], hd[0:1, 0:8], ba_row[:])
        e_row = T([1, 8], "e_row")
        nc.scalar.activation(out=e_row[:], in_=lg_row[:], func=AF.Exp)
        esum = T([1, 1], "esum")
        nc.vector.reduce_sum(esum[:], e_row[:], axis=AX.X)
        einv = T([1, 1], "einv")
        nc.vector.reciprocal(out=einv[:], in_=esum[:])
        pv = T([1, 9], "pv")
        nc.vector.tensor_scalar_mul(pv[0:1, 0:8], e_row[:], einv[0:1, 0:1])
        nc.vector.tensor_add(pv[0:1, 8:9], hd[0:1, 8:9], bc_sb[0:1, 0:1])
        nc.sync.dma_start(out=out_vec[0:1, 512:521], in_=pv[0:1, :])



def build(shard_rows=SHARD, n_cores=N_CORES):
    nc = bacc.Bacc(
        "TRN2",
        target_bir_lowering=False,
        debug=False,
        enable_asserts=False,
        num_devices=n_cores,
    )
    t = {}
    t["keys"] = nc.dram_tensor("keys", [shard_rows, 256], F32, kind="ExternalInput")
    t["vals"] = nc.dram_tensor("vals", [shard_rows, 256], F32, kind="ExternalInput")
    t["x_in"] = nc.dram_tensor("x_in", [256], F32, kind="ExternalInput")
    t["h_in"] = nc.dram_tensor("h_in", [256], F32, kind="ExternalInput")
    t["c_in"] = nc.dram_tensor("c_in", [256], F32, kind="ExternalInput")
    t["Wi_in"] = nc.dram_tensor("Wi_in", [1280, 256], F32, kind="ExternalInput")
    t["Wh_in"] = nc.dram_tensor("Wh_in", [1280, 256], F32, kind="ExternalInput")
    t["bias_in"] = nc.dram_tensor("bias_in", [1280], F32, kind="ExternalInput")
    t["Wa_in"] = nc.dram_tensor("Wa_in", [8, 256], F32, kind="ExternalInput")
    t["ba_in"] = nc.dram_tensor("ba_in", [8], F32, kind="ExternalInput")
    t["Wc_in"] = nc.dram_tensor("Wc_in", [1, 256], F32, kind="ExternalInput")
    t["bc_in"] = nc.dram_tensor("bc_in", [1], F32, kind="ExternalInput")
    t["out_vec"] = nc.dram_tensor("out_vec", [1, OUTW], F32, kind="ExternalOutput")

    with tile.TileContext(nc) as tc:
        _body(nc, tc, shard_rows, n_cores, t)
    nc.compile()
    return nc


def make_in_maps(x_t, h, c, keys, vals, Wi, bi, Wh, bh, Wa, ba, Wc, bc,
                 shard_rows=SHARD, n_cores=N_CORES):
    f = lambda a: np.ascontiguousarray(np.asarray(a, dtype=np.float32))
    keys = f(keys)
    vals = f(vals)
    common = {
        "x_in": f(x_t).reshape(256),
        "h_in": f(h).reshape(256),
        "c_in": f(c).reshape(256),
        "Wi_in": f(Wi),
        "Wh_in": f(Wh),
        "bias_in": f(bi) + f(bh),
        "Wa_in": f(Wa),
        "ba_in": f(ba),
        "Wc_in": f(Wc).reshape(1, 256),
        "bc_in": f(bc).reshape(1),
    }
    in_maps = []
    for i in range(n_cores):
        m = dict(common)
        m["keys"] = np.ascontiguousarray(keys[i * shard_rows:(i + 1) * shard_rows])
        m["vals"] = np.ascontiguousarray(vals[i * shard_rows:(i + 1) * shard_rows])
        in_maps.append(m)
    return in_maps


_NC_CACHE = None


def kernel(x_t, h, c, keys, vals, Wi, bi, Wh, bh, Wa, ba, Wc, bc, write_idx,
           _trace=False):
    global _NC_CACHE
    if _NC_CACHE is None:
        _NC_CACHE = build()
    nc = _NC_CACHE

    in_maps = make_in_maps(x_t, h, c, keys, vals, Wi, bi, Wh, bh, Wa, ba, Wc, bc)
    res = run_bass_kernel_spmd(nc, in_maps, core_ids=list(range(N_CORES)),
                               trace=_trace)
    out_vec = np.asarray(res.results[0]["out_vec"], dtype=np.float32)

    new_keys = np.array(np.asarray(keys, dtype=np.float32), copy=True)
    new_vals = np.array(np.asarray(vals, dtype=np.float32), copy=True)
    wi = int(write_idx)
    new_keys[wi] = out_vec[0, :H]
    new_vals[wi] = out_vec[0, H:2 * H]

    if _trace:
        kernel.last_exec_time_ns = res.exec_time_ns
        kernel.last_results = res
    return out_vec, new_keys, new_vals
